# revision 10
# baseline (speedup 1.0000x reference)
"""Two-layer GCN (GraphConv norm='both') as a Bass kernel on 8 trn2 NeuronCores.

Math (reference):
    ns = rsqrt(clip(out_deg(src),1));  nd = rsqrt(clip(in_deg(dst),1))
    C1  = (F * ns[:,None]) @ W1                     # [N,16]
    h1  = relu(M @ C1 * nd[:,None] + b1)            # M[d,s] = #edges s->d
    C2  = (h1 * ns[:,None]) @ W2                    # [N,7]
    out = M @ C2 * nd[:,None] + b2

Device formulation (per core c, owning rows rows_c = c*1250 ...):
    - F^T slice [K=10000, 1250] shipped pre-transposed (bf16), so the big
      GEMM streams F as the moving operand:  psum[16, m] += W1[kc]^T-chunk.
    - ns folded AFTER the GEMM (diagonal commutes):  C1s = C1 * ns.
    - Aggregation M @ X is a dense matmul against the host-built count
      matrix M^T [s, d] (bf16, exact small ints): psum[16, d] += C1s[sc]^T...
    - Two AllGathers exchange the tiny [N,16]/[N,7] activations.

Padding: k-dim 10000->10112 (79*128), per-core rows 1250->1280 (10*128),
gathered s-dim 8*1280=10240 (80*128). All pads are zeros; M^T pad rows are
zero so padded activations never contaminate real outputs.
"""

import sys

import numpy as np

try:
    import concourse  # noqa: F401
except ImportError:  # pragma: no cover
    sys.path.insert(0, "/opt/trn_rl_repo")

import concourse.bass as bass
import concourse.mybir as mybir
import concourse.tile as tile
from concourse import bacc
from concourse.bass_utils import run_bass_kernel_spmd
from concourse.masks import make_identity

BF16 = mybir.dt.bfloat16
F32 = mybir.dt.float32
U8 = mybir.dt.uint8
NPBF16 = mybir.dt.np(BF16)

N_NODES = 10000
HIDDEN = 16
LABELS = 7
NCORES = 8
ROWS = N_NODES // NCORES          # 1250 local rows per core
ROWS_PAD = 1280                   # 10 * 128
S_PAD = ROWS_PAD * NCORES         # 10240 gathered (padded) node dim
K_PAD = 10112                     # 79 * 128 contraction dim for F
KC = K_PAD // 128                 # 79 k-chunks
SC = S_PAD // 128                 # 80 s-chunks
MBLOCKS = [(0, 512), (512, 512), (1024, 256)]   # free-dim blocks of 1280
FGRP = 6                          # F k-chunks per DMA group
MGRP = 4                          # MT s-chunks per cast group


def _grouped(total, g):
    out, i = [], 0
    while i < total:
        out.append((i, min(g, total - i)))
        i += g
    return out


def build_nc():
    nc = bacc.Bacc(None, target_bir_lowering=False, debug=False)

    ft = nc.declare_dram_parameter("ft", [K_PAD, ROWS_PAD], BF16, isOutput=False)
    mt = nc.declare_dram_parameter("mt", [S_PAD, ROWS_PAD], U8, isOutput=False)
    w1 = nc.declare_dram_parameter("w1", [K_PAD, HIDDEN], BF16, isOutput=False)
    w2 = nc.declare_dram_parameter("w2", [HIDDEN, LABELS], BF16, isOutput=False)
    ns16 = nc.declare_dram_parameter("ns16", [HIDDEN, ROWS_PAD], F32, isOutput=False)
    nd16 = nc.declare_dram_parameter("nd16", [HIDDEN, ROWS_PAD], F32, isOutput=False)
    nd7 = nc.declare_dram_parameter("nd7", [LABELS, ROWS_PAD], F32, isOutput=False)
    b1c = nc.declare_dram_parameter("b1c", [HIDDEN, 1], F32, isOutput=False)
    b2c = nc.declare_dram_parameter("b2c", [LABELS, 1], F32, isOutput=False)
    out = nc.declare_dram_parameter("out", [ROWS_PAD, LABELS], F32, isOutput=True)

    rg = [list(range(NCORES))]

    with tile.TileContext(nc) as tc:
        with (
            tc.tile_pool(name="dram", bufs=1, space=bass.MemorySpace.DRAM) as dpool,
            tc.tile_pool(name="const", bufs=1) as cpool,
            tc.tile_pool(name="fpool", bufs=2) as fpool,
            tc.tile_pool(name="mpool", bufs=2) as mpool,
            tc.tile_pool(name="wk", bufs=2) as wk,
            tc.tile_pool(name="psG", bufs=3, space=bass.MemorySpace.PSUM) as psG,
            tc.tile_pool(name="psS", bufs=2, space=bass.MemorySpace.PSUM) as psS,
        ):
            # collective bounce buffers (internal DRAM; outs must be Shared)
            ag1_in = dpool.tile([ROWS_PAD, HIDDEN], BF16, tag="ag1i")
            ag1_out = dpool.tile([S_PAD, HIDDEN], BF16, tag="ag1o", addr_space="Shared")
            ag2_in = dpool.tile([ROWS_PAD, LABELS], BF16, tag="ag2i")
            ag2_out = dpool.tile([S_PAD, LABELS], BF16, tag="ag2o", addr_space="Shared")

            # ---- constants ----
            w1_sb = cpool.tile([128, KC, HIDDEN], BF16, tag="w1")
            nc.scalar.dma_start(
                w1_sb[:], w1[:, :].rearrange("(c p) f -> p c f", p=128)
            )
            w2_sb = cpool.tile([HIDDEN, LABELS], BF16, tag="w2")
            nc.scalar.dma_start(w2_sb[:], w2[:, :])
            ns16_sb = cpool.tile([HIDDEN, ROWS_PAD], F32, tag="ns16")
            nc.scalar.dma_start(ns16_sb[:], ns16[:, :])
            nd16_sb = cpool.tile([HIDDEN, ROWS_PAD], F32, tag="nd16")
            nc.scalar.dma_start(nd16_sb[:], nd16[:, :])
            nd7_sb = cpool.tile([LABELS, ROWS_PAD], F32, tag="nd7")
            nc.scalar.dma_start(nd7_sb[:], nd7[:, :])
            b1_sb = cpool.tile([HIDDEN, 1], F32, tag="b1")
            nc.scalar.dma_start(b1_sb[:], b1c[:, :])
            b2_sb = cpool.tile([LABELS, 1], F32, tag="b2")
            nc.scalar.dma_start(b2_sb[:], b2c[:, :])
            id_b = cpool.tile([HIDDEN, HIDDEN], BF16, tag="idb")
            make_identity(nc, id_b[:])
            id_f = cpool.tile([LABELS, LABELS], F32, tag="idf")
            make_identity(nc, id_f[:])

            c1sT = cpool.tile([HIDDEN, ROWS_PAD], BF16, tag="c1sT")
            h1sT = cpool.tile([HIDDEN, ROWS_PAD], BF16, tag="h1sT")
            outT = cpool.tile([LABELS, ROWS_PAD], F32, tag="outT")
            c1nat = cpool.tile([128, 10, HIDDEN], BF16, tag="c1nat")
            c2nat = cpool.tile([128, 10, LABELS], BF16, tag="c2nat")
            onat = cpool.tile([128, 10, LABELS], F32, tag="onat")
            c1_full = cpool.tile([128, SC, HIDDEN], BF16, tag="c1f")
            c2_full = cpool.tile([128, SC, LABELS], BF16, tag="c2f")
            # resident uint8 M^T  (~100KB/partition)
            mtres = cpool.tile([128, SC, ROWS_PAD], U8, tag="mtres")

            # ---- phase A: C1s^T[16, m] = (W1^T @ F^T_c) * ns  (4x col-tiled) ----
            ps_c1 = [
                psG.tile([128, msz], F32, tag="gemm", name=f"psc1_{i}")
                for i, (_, msz) in enumerate(MBLOCKS)
            ]
            for g0, gsz in _grouped(KC, FGRP):
                fbuf = fpool.tile([128, gsz, ROWS_PAD], BF16, tag="fbuf")
                nc.scalar.dma_start(
                    fbuf[:],
                    ft[g0 * 128 : (g0 + gsz) * 128, :].rearrange(
                        "(c p) f -> p c f", p=128
                    ),
                )
                for j in range(gsz):
                    kc = g0 + j
                    g = kc % 4
                    for mb, (m0, msz) in enumerate(MBLOCKS):
                        nc.tensor.matmul(
                            ps_c1[mb][32 * g : 32 * g + HIDDEN, :],
                            w1_sb[:, kc, :],
                            fbuf[:, j, m0 : m0 + msz],
                            start=(kc < 4),
                            stop=(kc >= KC - 4),
                            tile_position=(0, 32 * g),
                        )
            # MT resident load (after F in program order -> drains after F DMAs)
            MT_PIECES = 4
            psc = SC // MT_PIECES
            for p in range(MT_PIECES):
                nc.scalar.dma_start(
                    mtres[:, p * psc : (p + 1) * psc, :],
                    mt[p * psc * 128 : (p + 1) * psc * 128, :].rearrange(
                        "(c p) f -> p c f", p=128
                    ),
                )
            # strip-sum + scale
            for mb, (m0, msz) in enumerate(MBLOCKS):
                acc = wk.tile([HIDDEN, msz], F32, tag="acc")
                nc.vector.tensor_copy(acc[:], ps_c1[mb][0:HIDDEN, :])
                for g in range(1, 4):
                    nc.vector.tensor_tensor(
                        acc[:], acc[:], ps_c1[mb][32 * g : 32 * g + HIDDEN, :],
                        op=mybir.AluOpType.add,
                    )
                nc.vector.tensor_tensor(
                    c1sT[:, m0 : m0 + msz], acc[:], ns16_sb[:, m0 : m0 + msz],
                    op=mybir.AluOpType.mult,
                )

            # transpose C1s^T -> natural [1280, 16], stage and gather
            for t in range(10):
                pt = psS.tile([128, HIDDEN], BF16, tag="tr")
                nc.tensor.transpose(
                    pt[:], c1sT[:, t * 128 : (t + 1) * 128], id_b[:]
                )
                nc.vector.tensor_copy(c1nat[:, t, :], pt[:])
            nc.gpsimd.dma_start(
                ag1_in[:].rearrange("(t p) f -> p t f", p=128), c1nat[:]
            )
            nc.gpsimd.collective_compute(
                "AllGather",
                mybir.AluOpType.bypass,
                ins=[ag1_in.opt()],
                outs=[ag1_out.opt()],
                replica_groups=rg,
            )
            nc.gpsimd.dma_start(
                c1_full[:], ag1_out[:].rearrange("(c p) f -> p c f", p=128)
            )

            def agg_pass(full_sb, width, post):
                """agg[mb] strips = sum_sc full_sb[:,sc,:].T @ MT_bf[sc, mblock]."""
                ps_l = [
                    psG.tile([128, msz], F32, tag="gemm", name=f"psl_{i}")
                    for i, (_, msz) in enumerate(MBLOCKS)
                ]
                for gi, (g0, gsz) in enumerate(_grouped(SC, MGRP)):
                    mbuf = mpool.tile([128, gsz, ROWS_PAD], BF16, tag="mbuf")
                    eng = nc.vector if gi % 2 == 0 else nc.gpsimd
                    eng.tensor_copy(mbuf[:], mtres[:, g0 : g0 + gsz, :])
                    for j in range(gsz):
                        sc = g0 + j
                        g = sc % 4
                        for mb, (m0, msz) in enumerate(MBLOCKS):
                            nc.tensor.matmul(
                                ps_l[mb][32 * g : 32 * g + width, :],
                                full_sb[:, sc, :],
                                mbuf[:, j, m0 : m0 + msz],
                                start=(sc < 4),
                                stop=(sc >= SC - 4),
                                tile_position=(0, 32 * g),
                            )
                for mb, (m0, msz) in enumerate(MBLOCKS):
                    acc = wk.tile([width, msz], F32, tag="acc")
                    nc.vector.tensor_copy(acc[:], ps_l[mb][0:width, :])
                    for g in range(1, 4):
                        nc.vector.tensor_tensor(
                            acc[:], acc[:], ps_l[mb][32 * g : 32 * g + width, :],
                            op=mybir.AluOpType.add,
                        )
                    post(mb, m0, msz, acc)

            # ---- phase B: agg1 -> h1s^T -> C2^T ----
            def post1(mb, m0, msz, acc):
                nc.vector.tensor_tensor(
                    acc[:], acc[:], nd16_sb[:, m0 : m0 + msz], op=mybir.AluOpType.mult
                )
                tmp2 = wk.tile([HIDDEN, msz], F32, tag="tmp2")
                nc.scalar.activation(
                    tmp2[:], acc[:], mybir.ActivationFunctionType.Relu,
                    bias=b1_sb[:, 0:1], scale=1.0,
                )
                nc.vector.tensor_tensor(
                    h1sT[:, m0 : m0 + msz], tmp2[:], ns16_sb[:, m0 : m0 + msz],
                    op=mybir.AluOpType.mult,
                )
                ps2 = psS.tile([LABELS, msz], F32, tag="tr")
                nc.tensor.matmul(
                    ps2[:], w2_sb[:], h1sT[:, m0 : m0 + msz], start=True, stop=True
                )
                nc.vector.tensor_copy(outT[:, m0 : m0 + msz], ps2[:])

            agg_pass(c1_full, HIDDEN, post1)

            # outT currently holds C2^T (f32); transpose to natural bf16
            for t in range(10):
                pt = psS.tile([128, HIDDEN], BF16, tag="tr")
                ctmp = wk.tile([LABELS, 128], BF16, tag="ctmp")
                nc.vector.tensor_copy(ctmp[:], outT[:, t * 128 : (t + 1) * 128])
                nc.tensor.transpose(
                    pt[0:128, 0:LABELS], ctmp[:], id_b[0:LABELS, 0:LABELS]
                )
                nc.vector.tensor_copy(c2nat[:, t, :], pt[0:128, 0:LABELS])
            nc.gpsimd.dma_start(
                ag2_in[:].rearrange("(t p) f -> p t f", p=128), c2nat[:]
            )
            nc.gpsimd.collective_compute(
                "AllGather",
                mybir.AluOpType.bypass,
                ins=[ag2_in.opt()],
                outs=[ag2_out.opt()],
                replica_groups=rg,
            )
            nc.gpsimd.dma_start(
                c2_full[:], ag2_out[:].rearrange("(c p) f -> p c f", p=128)
            )

            # ---- phase C: agg2 -> out ----
            def post2(mb, m0, msz, acc):
                nc.vector.tensor_tensor(
                    acc[:], acc[:], nd7_sb[:, m0 : m0 + msz], op=mybir.AluOpType.mult
                )
                nc.vector.tensor_scalar_add(
                    outT[:, m0 : m0 + msz], acc[:], b2_sb[:, 0:1]
                )

            agg_pass(c2_full, LABELS, post2)

            for t in range(10):
                pt = psS.tile([128, HIDDEN], F32, tag="tr")
                nc.tensor.transpose(
                    pt[0:128, 0:LABELS],
                    outT[:, t * 128 : (t + 1) * 128],
                    id_f[:],
                )
                nc.vector.tensor_copy(onat[:, t, :], pt[0:128, 0:LABELS])
            nc.scalar.dma_start(
                out[:, :].rearrange("(t p) f -> p t f", p=128), onat[:]
            )

    nc.compile()
    return nc


_NC = None


def _get_nc():
    global _NC
    if _NC is None:
        _NC = build_nc()
    return _NC


def _prepare_in_maps(features, src, dst, W1, b1, W2, b2):
    f = np.asarray(features, dtype=np.float32)
    src = np.asarray(src).astype(np.int64)
    dst = np.asarray(dst).astype(np.int64)
    W1 = np.asarray(W1, dtype=np.float32)
    b1 = np.asarray(b1, dtype=np.float32)
    W2 = np.asarray(W2, dtype=np.float32)
    b2 = np.asarray(b2, dtype=np.float32)

    out_deg = np.bincount(src, minlength=N_NODES).astype(np.float32)
    in_deg = np.bincount(dst, minlength=N_NODES).astype(np.float32)
    ns = 1.0 / np.sqrt(np.clip(out_deg, 1.0, None))
    nd = 1.0 / np.sqrt(np.clip(in_deg, 1.0, None))

    # M^T[s_padded, d] edge-count matrix, padded-s indexing: s=(c, r)->c*1280+r
    s_pad_idx = (src // ROWS) * ROWS_PAD + (src % ROWS)
    mt_counts = np.zeros((S_PAD, N_NODES), dtype=np.uint8)
    np.add.at(mt_counts, (s_pad_idx, dst), 1)

    w1p = np.zeros((K_PAD, HIDDEN), dtype=NPBF16)
    w1p[:N_NODES] = W1.astype(NPBF16)
    w2b = W2.astype(NPBF16)
    b1c = b1.reshape(HIDDEN, 1)
    b2c = b2.reshape(LABELS, 1)

    in_maps = []
    for c in range(NCORES):
        r0 = c * ROWS
        ft_c = np.zeros((K_PAD, ROWS_PAD), dtype=NPBF16)
        ft_c[:N_NODES, :ROWS] = f[r0 : r0 + ROWS, :].T.astype(NPBF16)
        mt_c = np.zeros((S_PAD, ROWS_PAD), dtype=np.uint8)
        mt_c[:, :ROWS] = mt_counts[:, r0 : r0 + ROWS]
        ns_c = np.zeros((1, ROWS_PAD), dtype=np.float32)
        ns_c[0, :ROWS] = ns[r0 : r0 + ROWS]
        nd_c = np.zeros((1, ROWS_PAD), dtype=np.float32)
        nd_c[0, :ROWS] = nd[r0 : r0 + ROWS]
        in_maps.append(
            {
                "ft": ft_c,
                "mt": mt_c,
                "w1": w1p,
                "w2": w2b,
                "ns16": np.repeat(ns_c, HIDDEN, axis=0),
                "nd16": np.repeat(nd_c, HIDDEN, axis=0),
                "nd7": np.repeat(nd_c, LABELS, axis=0),
                "b1c": b1c,
                "b2c": b2c,
            }
        )
    return in_maps


def run(inputs, trace=False):
    nc = _get_nc()
    in_maps = _prepare_in_maps(**inputs)
    res = run_bass_kernel_spmd(nc, in_maps, list(range(NCORES)), trace=trace)
    out = np.concatenate(
        [np.asarray(res.results[c]["out"])[:ROWS] for c in range(NCORES)], axis=0
    ).astype(np.float32)
    return out, res


def kernel(**inputs):
    out, _ = run(inputs, trace=False)
    return out


# revision 13
# speedup vs baseline: 1.7728x; 1.7728x over previous
"""Two-layer GCN (GraphConv norm='both') as a Bass kernel on 8 trn2 NeuronCores.

Math (reference):
    ns = rsqrt(clip(out_deg(src),1));  nd = rsqrt(clip(in_deg(dst),1))
    C1  = (F * ns[:,None]) @ W1                     # [N,16]
    h1  = relu(M @ C1 * nd[:,None] + b1)            # M[d,s] = #edges s->d
    C2  = (h1 * ns[:,None]) @ W2                    # [N,7]
    out = M @ C2 * nd[:,None] + b2

Device formulation (per core c, owning rows rows_c = c*1250 ...):
    - F^T slice [K=10000, 1250] shipped pre-transposed (bf16), so the big
      GEMM streams F as the moving operand:  psum[16, m] += W1[kc]^T-chunk.
    - ns folded AFTER the GEMM (diagonal commutes):  C1s = C1 * ns.
    - Aggregation M @ X is a dense matmul against the host-built count
      matrix M^T [s, d] (bf16, exact small ints): psum[16, d] += C1s[sc]^T...
    - Two AllGathers exchange the tiny [N,16]/[N,7] activations.

Padding: k-dim 10000->10112 (79*128), per-core rows 1250->1280 (10*128),
gathered s-dim 8*1280=10240 (80*128). All pads are zeros; M^T pad rows are
zero so padded activations never contaminate real outputs.
"""

import sys

import numpy as np

try:
    import concourse  # noqa: F401
except ImportError:  # pragma: no cover
    sys.path.insert(0, "/opt/trn_rl_repo")

import concourse.bass as bass
import concourse.mybir as mybir
import concourse.tile as tile
from concourse import bacc
from concourse.bass_utils import run_bass_kernel_spmd
from concourse.masks import make_identity

BF16 = mybir.dt.bfloat16
F32 = mybir.dt.float32
U8 = mybir.dt.uint8
NPBF16 = mybir.dt.np(BF16)

N_NODES = 10000
HIDDEN = 16
LABELS = 7
NCORES = 8
ROWS = N_NODES // NCORES          # 1250 local rows per core
ROWS_PAD = 1280                   # 10 * 128
S_PAD = ROWS_PAD * NCORES         # 10240 gathered (padded) node dim
K_PAD = 10112                     # 79 * 128 contraction dim for F
KC = K_PAD // 128                 # 79 k-chunks
SC = S_PAD // 128                 # 80 s-chunks
MBLOCKS = [(0, 512), (512, 512), (1024, 256)]   # free-dim blocks of 1280
FGRP = 6                          # F k-chunks per DMA group
MGRP = 6                          # MT s-chunks per DMA group
MT_U8 = False                     # ship M^T as uint8; SWDGE DMA casts to bf16 inline
RES_CH = 32                       # MT chunks kept SBUF-resident between passes


def _grouped(total, g):
    out, i = [], 0
    while i < total:
        out.append((i, min(g, total - i)))
        i += g
    return out


def build_nc():
    nc = bacc.Bacc(None, target_bir_lowering=False, debug=False)

    ft = nc.declare_dram_parameter("ft", [K_PAD, ROWS_PAD], BF16, isOutput=False)
    mt = nc.declare_dram_parameter("mt", [S_PAD, ROWS_PAD], U8 if MT_U8 else BF16, isOutput=False)
    w1 = nc.declare_dram_parameter("w1", [K_PAD, HIDDEN], BF16, isOutput=False)
    w2 = nc.declare_dram_parameter("w2", [HIDDEN, LABELS], BF16, isOutput=False)
    ns16 = nc.declare_dram_parameter("ns16", [HIDDEN, ROWS_PAD], F32, isOutput=False)
    nd16 = nc.declare_dram_parameter("nd16", [HIDDEN, ROWS_PAD], F32, isOutput=False)
    nd7 = nc.declare_dram_parameter("nd7", [LABELS, ROWS_PAD], F32, isOutput=False)
    b1c = nc.declare_dram_parameter("b1c", [HIDDEN, 1], F32, isOutput=False)
    b2c = nc.declare_dram_parameter("b2c", [LABELS, 1], F32, isOutput=False)
    out = nc.declare_dram_parameter("out", [ROWS_PAD, LABELS], F32, isOutput=True)

    rg = [list(range(NCORES))]

    with tile.TileContext(nc) as tc:
        with (
            tc.tile_pool(name="dram", bufs=1, space=bass.MemorySpace.DRAM) as dpool,
            tc.tile_pool(name="const", bufs=1) as cpool,
            tc.tile_pool(name="fpool", bufs=2) as fpool,
            tc.tile_pool(name="mpool", bufs=2) as mpool,
            tc.tile_pool(name="wk", bufs=2) as wk,
            tc.tile_pool(name="psG", bufs=3, space=bass.MemorySpace.PSUM) as psG,
            tc.tile_pool(name="psS", bufs=2, space=bass.MemorySpace.PSUM) as psS,
        ):
            # collective bounce buffers (internal DRAM; outs must be Shared)
            ag1_in = dpool.tile([ROWS_PAD, HIDDEN], BF16, tag="ag1i")
            ag1_out = dpool.tile([S_PAD, HIDDEN], BF16, tag="ag1o", addr_space="Shared")
            ag2_in = dpool.tile([ROWS_PAD, LABELS], BF16, tag="ag2i")
            ag2_out = dpool.tile([S_PAD, LABELS], BF16, tag="ag2o", addr_space="Shared")

            # ---- constants ----
            w1_sb = cpool.tile([128, KC, HIDDEN], BF16, tag="w1")
            nc.scalar.dma_start(
                w1_sb[:], w1[:, :].rearrange("(c p) f -> p c f", p=128)
            )
            w2_sb = cpool.tile([HIDDEN, LABELS], BF16, tag="w2")
            nc.scalar.dma_start(w2_sb[:], w2[:, :])
            ns16_sb = cpool.tile([HIDDEN, ROWS_PAD], F32, tag="ns16")
            nc.scalar.dma_start(ns16_sb[:], ns16[:, :])
            nd16_sb = cpool.tile([HIDDEN, ROWS_PAD], F32, tag="nd16")
            nc.scalar.dma_start(nd16_sb[:], nd16[:, :])
            nd7_sb = cpool.tile([LABELS, ROWS_PAD], F32, tag="nd7")
            nc.scalar.dma_start(nd7_sb[:], nd7[:, :])
            b1_sb = cpool.tile([HIDDEN, 1], F32, tag="b1")
            nc.scalar.dma_start(b1_sb[:], b1c[:, :])
            b2_sb = cpool.tile([LABELS, 1], F32, tag="b2")
            nc.scalar.dma_start(b2_sb[:], b2c[:, :])
            id_b = cpool.tile([HIDDEN, HIDDEN], BF16, tag="idb")
            make_identity(nc, id_b[:])
            id_f = cpool.tile([LABELS, LABELS], F32, tag="idf")
            make_identity(nc, id_f[:])

            c1sT = cpool.tile([HIDDEN, ROWS_PAD], BF16, tag="c1sT")
            h1sT = cpool.tile([HIDDEN, ROWS_PAD], BF16, tag="h1sT")
            outT = cpool.tile([LABELS, ROWS_PAD], F32, tag="outT")
            c1nat = cpool.tile([128, 10, HIDDEN], BF16, tag="c1nat")
            c2nat = cpool.tile([128, 10, LABELS], BF16, tag="c2nat")
            onat = cpool.tile([128, 10, LABELS], F32, tag="onat")
            c1_full = cpool.tile([128, SC, HIDDEN], BF16, tag="c1f")
            c2_full = cpool.tile([128, SC, LABELS], BF16, tag="c2f")
            # partial bf16 M^T residency: first RES_CH chunks stay on-chip
            mtres = cpool.tile([128, RES_CH, ROWS_PAD], BF16, tag="mtres")

            # ---- phase A: C1s^T[16, m] = (W1^T @ F^T_c) * ns  (4x col-tiled) ----
            ps_c1 = [
                psG.tile([128, msz], F32, tag="gemm", name=f"psc1_{i}")
                for i, (_, msz) in enumerate(MBLOCKS)
            ]
            for g0, gsz in _grouped(KC, FGRP):
                fbuf = fpool.tile([128, gsz, ROWS_PAD], BF16, tag="fbuf")
                nc.scalar.dma_start(
                    fbuf[:],
                    ft[g0 * 128 : (g0 + gsz) * 128, :].rearrange(
                        "(c p) f -> p c f", p=128
                    ),
                )
                for j in range(gsz):
                    kc = g0 + j
                    g = kc % 4
                    for mb, (m0, msz) in enumerate(MBLOCKS):
                        nc.tensor.matmul(
                            ps_c1[mb][32 * g : 32 * g + HIDDEN, :],
                            w1_sb[:, kc, :],
                            fbuf[:, j, m0 : m0 + msz],
                            start=(kc < 4),
                            stop=(kc >= KC - 4),
                            tile_position=(0, 32 * g),
                        )
            # strip-sum + scale
            for mb, (m0, msz) in enumerate(MBLOCKS):
                acc = wk.tile([HIDDEN, msz], F32, tag="acc")
                nc.vector.tensor_copy(acc[:], ps_c1[mb][0:HIDDEN, :])
                for g in range(1, 4):
                    nc.vector.tensor_tensor(
                        acc[:], acc[:], ps_c1[mb][32 * g : 32 * g + HIDDEN, :],
                        op=mybir.AluOpType.add,
                    )
                nc.vector.tensor_tensor(
                    c1sT[:, m0 : m0 + msz], acc[:], ns16_sb[:, m0 : m0 + msz],
                    op=mybir.AluOpType.mult,
                )

            # transpose C1s^T -> natural [1280, 16], stage and gather
            for t in range(10):
                pt = psS.tile([128, HIDDEN], BF16, tag="tr")
                nc.tensor.transpose(
                    pt[:], c1sT[:, t * 128 : (t + 1) * 128], id_b[:]
                )
                nc.vector.tensor_copy(c1nat[:, t, :], pt[:])
            nc.gpsimd.dma_start(
                ag1_in[:].rearrange("(t p) f -> p t f", p=128), c1nat[:]
            )
            nc.gpsimd.collective_compute(
                "AllGather",
                mybir.AluOpType.bypass,
                ins=[ag1_in.opt()],
                outs=[ag1_out.opt()],
                replica_groups=rg,
            )
            nc.gpsimd.dma_start(
                c1_full[:], ag1_out[:].rearrange("(c p) f -> p c f", p=128)
            )

            def agg_pass(full_sb, width, post, first):
                """agg[mb] strips = sum_sc full_sb[:,sc,:].T @ MT[sc, mblock].
                Pass 1 (first=True) DMAs chunks [0, RES_CH) into the resident
                tile; pass 2 reads those from SBUF and streams only the rest."""
                ps_l = [
                    psG.tile([128, msz], F32, tag="gemm", name=f"psl_{i}")
                    for i, (_, msz) in enumerate(MBLOCKS)
                ]

                def mm(sc, src_ap):
                    g = sc % 4
                    for mb, (m0, msz) in enumerate(MBLOCKS):
                        nc.tensor.matmul(
                            ps_l[mb][32 * g : 32 * g + width, :],
                            full_sb[:, sc, :],
                            src_ap[:, m0 : m0 + msz],
                            start=(sc < 4),
                            stop=(sc >= SC - 4),
                            tile_position=(0, 32 * g),
                        )

                mt_dma = nc.gpsimd.dma_start if MT_U8 else nc.scalar.dma_start
                if first:
                    for g0, gsz in _grouped(RES_CH, MGRP):
                        mt_dma(
                            mtres[:, g0 : g0 + gsz, :],
                            mt[g0 * 128 : (g0 + gsz) * 128, :].rearrange(
                                "(c p) f -> p c f", p=128
                            ),
                        )
                for g0, gsz in _grouped(RES_CH, MGRP):
                    for j in range(gsz):
                        mm(g0 + j, mtres[:, g0 + j, :])
                for g0, gsz in _grouped(SC - RES_CH, MGRP):
                    g0 += RES_CH
                    mbuf = mpool.tile([128, gsz, ROWS_PAD], BF16, tag="mbuf")
                    mt_dma(
                        mbuf[:],
                        mt[g0 * 128 : (g0 + gsz) * 128, :].rearrange(
                            "(c p) f -> p c f", p=128
                        ),
                    )
                    for j in range(gsz):
                        mm(g0 + j, mbuf[:, j, :])
                for mb, (m0, msz) in enumerate(MBLOCKS):
                    acc = wk.tile([width, msz], F32, tag="acc")
                    nc.vector.tensor_copy(acc[:], ps_l[mb][0:width, :])
                    for g in range(1, 4):
                        nc.vector.tensor_tensor(
                            acc[:], acc[:], ps_l[mb][32 * g : 32 * g + width, :],
                            op=mybir.AluOpType.add,
                        )
                    post(mb, m0, msz, acc)

            # ---- phase B: agg1 -> h1s^T -> C2^T ----
            def post1(mb, m0, msz, acc):
                nc.vector.tensor_tensor(
                    acc[:], acc[:], nd16_sb[:, m0 : m0 + msz], op=mybir.AluOpType.mult
                )
                tmp2 = wk.tile([HIDDEN, msz], F32, tag="tmp2")
                nc.scalar.activation(
                    tmp2[:], acc[:], mybir.ActivationFunctionType.Relu,
                    bias=b1_sb[:, 0:1], scale=1.0,
                )
                nc.vector.tensor_tensor(
                    h1sT[:, m0 : m0 + msz], tmp2[:], ns16_sb[:, m0 : m0 + msz],
                    op=mybir.AluOpType.mult,
                )
                ps2 = psS.tile([LABELS, msz], F32, tag="tr")
                nc.tensor.matmul(
                    ps2[:], w2_sb[:], h1sT[:, m0 : m0 + msz], start=True, stop=True
                )
                nc.vector.tensor_copy(outT[:, m0 : m0 + msz], ps2[:])

            agg_pass(c1_full, HIDDEN, post1, first=True)

            # outT currently holds C2^T (f32); transpose to natural bf16
            for t in range(10):
                pt = psS.tile([128, HIDDEN], BF16, tag="tr")
                ctmp = wk.tile([LABELS, 128], BF16, tag="ctmp")
                nc.vector.tensor_copy(ctmp[:], outT[:, t * 128 : (t + 1) * 128])
                nc.tensor.transpose(
                    pt[0:128, 0:LABELS], ctmp[:], id_b[0:LABELS, 0:LABELS]
                )
                nc.vector.tensor_copy(c2nat[:, t, :], pt[0:128, 0:LABELS])
            nc.gpsimd.dma_start(
                ag2_in[:].rearrange("(t p) f -> p t f", p=128), c2nat[:]
            )
            nc.gpsimd.collective_compute(
                "AllGather",
                mybir.AluOpType.bypass,
                ins=[ag2_in.opt()],
                outs=[ag2_out.opt()],
                replica_groups=rg,
            )
            nc.gpsimd.dma_start(
                c2_full[:], ag2_out[:].rearrange("(c p) f -> p c f", p=128)
            )

            # ---- phase C: agg2 -> out ----
            def post2(mb, m0, msz, acc):
                nc.vector.tensor_tensor(
                    acc[:], acc[:], nd7_sb[:, m0 : m0 + msz], op=mybir.AluOpType.mult
                )
                nc.vector.tensor_scalar_add(
                    outT[:, m0 : m0 + msz], acc[:], b2_sb[:, 0:1]
                )

            agg_pass(c2_full, LABELS, post2, first=False)

            for t in range(10):
                pt = psS.tile([128, HIDDEN], F32, tag="tr")
                nc.tensor.transpose(
                    pt[0:128, 0:LABELS],
                    outT[:, t * 128 : (t + 1) * 128],
                    id_f[:],
                )
                nc.vector.tensor_copy(onat[:, t, :], pt[0:128, 0:LABELS])
            nc.scalar.dma_start(
                out[:, :].rearrange("(t p) f -> p t f", p=128), onat[:]
            )

    nc.compile()
    return nc


_NC = None


def _get_nc():
    global _NC
    if _NC is None:
        _NC = build_nc()
    return _NC


def _prepare_in_maps(features, src, dst, W1, b1, W2, b2):
    f = np.asarray(features, dtype=np.float32)
    src = np.asarray(src).astype(np.int64)
    dst = np.asarray(dst).astype(np.int64)
    W1 = np.asarray(W1, dtype=np.float32)
    b1 = np.asarray(b1, dtype=np.float32)
    W2 = np.asarray(W2, dtype=np.float32)
    b2 = np.asarray(b2, dtype=np.float32)

    out_deg = np.bincount(src, minlength=N_NODES).astype(np.float32)
    in_deg = np.bincount(dst, minlength=N_NODES).astype(np.float32)
    ns = 1.0 / np.sqrt(np.clip(out_deg, 1.0, None))
    nd = 1.0 / np.sqrt(np.clip(in_deg, 1.0, None))

    # M^T[s_padded, d] edge-count matrix, padded-s indexing: s=(c, r)->c*1280+r
    s_pad_idx = (src // ROWS) * ROWS_PAD + (src % ROWS)
    mt_counts = np.zeros((S_PAD, N_NODES), dtype=np.uint8)
    np.add.at(mt_counts, (s_pad_idx, dst), 1)

    w1p = np.zeros((K_PAD, HIDDEN), dtype=NPBF16)
    w1p[:N_NODES] = W1.astype(NPBF16)
    w2b = W2.astype(NPBF16)
    b1c = b1.reshape(HIDDEN, 1)
    b2c = b2.reshape(LABELS, 1)

    in_maps = []
    for c in range(NCORES):
        r0 = c * ROWS
        ft_c = np.zeros((K_PAD, ROWS_PAD), dtype=NPBF16)
        ft_c[:N_NODES, :ROWS] = f[r0 : r0 + ROWS, :].T.astype(NPBF16)
        mtdt = np.uint8 if MT_U8 else NPBF16
        mt_c = np.zeros((S_PAD, ROWS_PAD), dtype=mtdt)
        mt_c[:, :ROWS] = mt_counts[:, r0 : r0 + ROWS].astype(mtdt)
        ns_c = np.zeros((1, ROWS_PAD), dtype=np.float32)
        ns_c[0, :ROWS] = ns[r0 : r0 + ROWS]
        nd_c = np.zeros((1, ROWS_PAD), dtype=np.float32)
        nd_c[0, :ROWS] = nd[r0 : r0 + ROWS]
        in_maps.append(
            {
                "ft": ft_c,
                "mt": mt_c,
                "w1": w1p,
                "w2": w2b,
                "ns16": np.repeat(ns_c, HIDDEN, axis=0),
                "nd16": np.repeat(nd_c, HIDDEN, axis=0),
                "nd7": np.repeat(nd_c, LABELS, axis=0),
                "b1c": b1c,
                "b2c": b2c,
            }
        )
    return in_maps


def run(inputs, trace=False):
    nc = _get_nc()
    in_maps = _prepare_in_maps(**inputs)
    res = run_bass_kernel_spmd(nc, in_maps, list(range(NCORES)), trace=trace)
    out = np.concatenate(
        [np.asarray(res.results[c]["out"])[:ROWS] for c in range(NCORES)], axis=0
    ).astype(np.float32)
    return out, res


def kernel(**inputs):
    out, _ = run(inputs, trace=False)
    return out
